# revision 20
# baseline (speedup 1.0000x reference)
"""Trainium2 Bass kernel for nn_FC_89094801588783.

Computes, for x[B=16, N=8192, Fin=256], W[256,256], b[256], gamma[256], beta[256]:
    y = x @ W.T + b                       (per-token Linear)
    per-sample BatchNorm over N (biased var), then gamma/beta affine.

Key structural choices vs a straightforward port:
  - bf16 end-to-end on device (x, W, y storage, output). The harness gate is
    rel_err < 2e-2; the bf16 pipeline measures ~5e-3. Halves HBM traffic.
  - The host pre-transposes x to [sample, Fin, N] and post-transposes the
    output, so the device never runs a PE transpose: the only PE work is the
    W-stationary matmul producing y^T[Fout, tok] directly.
  - The Linear bias b cancels exactly in BatchNorm (mean subtraction), so it
    is never sent to the device.
  - Per 2048-token batch: 16 matmuls accumulate into 8 PSUM banks; ACT
    evacuates PSUM->SBUF bf16 with accum_out (per-feature sum -> mean); DVE
    squares the bf16 y with accum_out (sum sq -> var). The normalize
    (y*k + shift) runs as DVE tensor_scalar ops, which hit the 4x DVE mode
    on all-bf16 operands (~0.8us per [128,2048] vs ~2us elsewhere), with a
    couple on ACT to balance the queues. GpSimd is kept off the big ops:
    concurrent Pool traffic poisons the shared SBUF ports and halves DVE
    throughput.
  - The per-sample finalize (mean/var -> k, shift) runs on DVE in its idle
    gap between the two samples' windows, with a single Sqrt hop on ACT.
  - DMAs are spread over both HWDGE queues (inputs on qSP; consts, store
    DMAs and half the tail on qAct) so transfers overlap instead of
    serializing on one ring.

Sharding: data-parallel over B across 8 NeuronCores (2 samples per core).
"""
import sys

sys.path.insert(0, "/opt/trn_rl_repo")

import numpy as np
import ml_dtypes

_NC_CACHE = {}

B, N, F = 16, 8192, 256
CORES = 8
SPB = B // CORES          # samples per core = 2
P = 128
NBATCH = 4                # token batches per sample
BT = N // NBATCH          # tokens per batch = 2048
NG = BT // 512            # 512-token PSUM groups per batch = 4
EPS = 1e-5
INV_N = 1.0 / N


def _build_nc():
    import concourse.bacc as bacc
    import concourse.tile as tile
    from concourse import mybir

    f32 = mybir.dt.float32
    bf16 = mybir.dt.bfloat16
    AF = mybir.ActivationFunctionType
    OP = mybir.AluOpType
    AX = mybir.AxisListType

    nc = bacc.Bacc("TRN2")
    x_d = nc.dram_tensor("x", [SPB, F, N], bf16, kind="ExternalInput")
    wt_d = nc.dram_tensor("wt", [F, F], bf16, kind="ExternalInput")
    g_d = nc.dram_tensor("gamma", [F], f32, kind="ExternalInput")
    be_d = nc.dram_tensor("beta", [F], f32, kind="ExternalInput")
    out_d = nc.dram_tensor("out", [SPB, F, N], bf16, kind="ExternalOutput")

    with tile.TileContext(nc) as tc:
        with (
            tc.tile_pool(name="consts", bufs=1) as consts,
            tc.tile_pool(name="xin", bufs=3) as xin,
            tc.tile_pool(name="xin0", bufs=4) as xin0,
            tc.tile_pool(name="ystore", bufs=1) as ystore,
            tc.tile_pool(name="acc", bufs=1) as accp,
            tc.tile_pool(name="fin", bufs=1) as finp,
            tc.tile_pool(name="scr", bufs=2) as scr,
            tc.tile_pool(name="outp", bufs=10) as outp,
            tc.tile_pool(name="ps", bufs=2, space="PSUM") as ps,
        ):
            # ---------------- constants (qAct queue; x input rides qSP) ----
            wT = consts.tile([P, 2, F], bf16)
            nc.scalar.dma_start(out=wT[:], in_=wt_d.rearrange("(c p) o -> p c o", p=P))
            g_col = consts.tile([P, 2], f32)
            nc.scalar.dma_start(out=g_col[:], in_=g_d.rearrange("(h p) -> p h", p=P))
            be_col = consts.tile([P, 2], f32)
            nc.scalar.dma_start(out=be_col[:], in_=be_d.rearrange("(h p) -> p h", p=P))

            # per-(sample, a) persistent state
            y_sb = [[None] * 2 for _ in range(SPB)]
            sums_t = [[None] * 2 for _ in range(SPB)]
            sq_t = [[None] * 2 for _ in range(SPB)]
            k_col = [[None] * 2 for _ in range(SPB)]
            sh_col = [[None] * 2 for _ in range(SPB)]
            for s in range(SPB):
                for a in range(2):
                    y_sb[s][a] = ystore.tile(
                        [P, NBATCH, BT], bf16, tag=f"y{s}{a}", name=f"y{s}{a}"
                    )
                    sums_t[s][a] = accp.tile(
                        [P, NBATCH], f32, tag=f"sm{s}{a}", name=f"sm{s}{a}")
                    sq_t[s][a] = accp.tile(
                        [P, NBATCH], f32, tag=f"sq{s}{a}", name=f"sq{s}{a}")

            xin_t = {}

            def emit_xin(s, j):
                if s == 0 and j == 0:
                    # first batch in 4 chunks so the first matmul starts early
                    parts = []
                    for g in range(NG):
                        t = xin0.tile([P, 2, 512], bf16, tag="x0", name=f"x0{g}")
                        eng = nc.sync if g % 2 == 0 else nc.scalar
                        eng.dma_start(
                            out=t[:],
                            in_=x_d[0, :, g * 512:(g + 1) * 512].rearrange(
                                "(c p) t -> p c t", p=P),
                        )
                        parts.append(t)
                    xin_t[(s, j)] = parts
                else:
                    t = xin.tile([P, 2, BT], bf16, tag="x")
                    nc.sync.dma_start(
                        out=t[:],
                        in_=x_d[s, :, j * BT:(j + 1) * BT].rearrange(
                            "(c p) t -> p c t", p=P),
                    )
                    xin_t[(s, j)] = t

            def emit_batch(s, j):
                x_t = xin_t.pop((s, j))
                nxt = s * NBATCH + j + 2
                if nxt < SPB * NBATCH:
                    emit_xin(nxt // NBATCH, nxt % NBATCH)

                pst = [ps.tile([P, NG, 512], f32, tag="ps", name=f"ps{s}{j}{a}")
                       for a in range(2)]
                # c outermost so 4 consecutive matmuls share one stationary
                for c in range(2):
                    for a in range(2):
                        for g in range(NG):
                            rhs = (x_t[g][:, c, :] if isinstance(x_t, list)
                                   else x_t[:, c, g * 512:(g + 1) * 512])
                            nc.tensor.matmul(
                                pst[a][:, g, :],
                                wT[:, c, a * P:(a + 1) * P],
                                rhs,
                                start=(c == 0), stop=(c == 1),
                            )
                for a in range(2):
                    # PSUM -> SBUF bf16; accumulator gives sum(y) per feature
                    nc.scalar.activation(
                        out=y_sb[s][a][:, j, :],
                        in_=pst[a].rearrange("p g t -> p (g t)"),
                        func=AF.Copy,
                        accum_out=sums_t[s][a][:, j:j + 1],
                    )
                for a in range(2):
                    # sum(y^2) from the bf16 y (walrus allows only one PSUM
                    # operand per DVE op, so this reads the evac result)
                    sq_scr = scr.tile([P, BT], bf16, tag="scr")
                    nc.vector.scalar_tensor_tensor(
                        out=sq_scr[:],
                        in0=y_sb[s][a][:, j, :],
                        scalar=1.0,
                        in1=y_sb[s][a][:, j, :],
                        op0=OP.mult,
                        op1=OP.mult,
                        accum_out=sq_t[s][a][:, j:j + 1],
                    )

            def emit_finalize(s):
                # DVE: two reduces + one reciprocal; everything else on ACT,
                # which has an idle gap at the window transition. Keeping the
                # chain off the deep DVE queue avoids ~2.3us bypass stalls
                # per dependency hop.
                for a in range(2):
                    S = finp.tile([P, 1], f32, tag=f"S{s}{a}", name=f"S{s}{a}")
                    nc.vector.tensor_reduce(
                        out=S[:], in_=sums_t[s][a][:], axis=AX.X, op=OP.add)
                    Q = finp.tile([P, 1], f32, tag=f"Q{s}{a}", name=f"Q{s}{a}")
                    nc.vector.tensor_reduce(
                        out=Q[:], in_=sq_t[s][a][:], axis=AX.X, op=OP.add)
                    mean = finp.tile([P, 1], f32, tag=f"mn{s}{a}", name=f"mn{s}{a}")
                    nc.scalar.activation(
                        out=mean[:], in_=S[:], func=AF.Copy, scale=INV_N)
                    m2 = finp.tile([P, 1], f32, tag=f"m2{s}{a}", name=f"m2{s}{a}")
                    nc.scalar.activation(
                        out=m2[:], in_=mean[:], func=AF.Square)
                    nm2 = finp.tile([P, 1], f32, tag=f"nm{s}{a}", name=f"nm{s}{a}")
                    nc.scalar.activation(
                        out=nm2[:], in_=m2[:], func=AF.Copy, scale=-1.0, bias=EPS)
                    vare = finp.tile([P, 1], f32, tag=f"vr{s}{a}", name=f"vr{s}{a}")
                    nc.scalar.activation(
                        out=vare[:], in_=Q[:], func=AF.Identity, scale=INV_N,
                        bias=nm2[:])
                    ivar = finp.tile([P, 1], f32, tag=f"iv{s}{a}", name=f"iv{s}{a}")
                    nc.vector.reciprocal(out=ivar[:], in_=vare[:])
                    k0 = finp.tile([P, 1], f32, tag=f"k0{s}{a}", name=f"k0{s}{a}")
                    nc.scalar.activation(
                        out=k0[:], in_=ivar[:], func=AF.Sqrt, bias=0.0, scale=1.0)
                    k = finp.tile([P, 1], f32, tag=f"k{s}{a}", name=f"k{s}{a}")
                    nc.scalar.activation(
                        out=k[:], in_=k0[:], func=AF.Copy,
                        scale=g_col[:, a:a + 1])
                    tmp = finp.tile([P, 1], f32, tag=f"tp{s}{a}", name=f"tp{s}{a}")
                    nc.scalar.activation(
                        out=tmp[:], in_=mean[:], func=AF.Copy, scale=k[:])
                    sh = finp.tile([P, 1], f32, tag=f"sh{s}{a}", name=f"sh{s}{a}")
                    nc.scalar.activation(
                        out=sh[:], in_=tmp[:], func=AF.Identity, scale=-1.0,
                        bias=be_col[:, a:a + 1])
                    k_col[s][a] = k
                    sh_col[s][a] = sh

            def emit_norm(s, a, j, eng, dma_eng):
                osb = outp.tile([P, BT], bf16, tag="o")
                if eng == "act":
                    nc.scalar.activation(
                        out=osb[:], in_=y_sb[s][a][:, j, :], func=AF.Identity,
                        bias=sh_col[s][a][:], scale=k_col[s][a][:],
                    )
                else:
                    e = nc.vector if eng == "dve" else nc.gpsimd
                    e.tensor_scalar(
                        out=osb[:], in0=y_sb[s][a][:, j, :],
                        scalar1=k_col[s][a][:], scalar2=sh_col[s][a][:],
                        op0=OP.mult, op1=OP.add,
                    )
                dma_eng.dma_start(
                    out=out_d[s, a * P:(a + 1) * P, j * BT:(j + 1) * BT],
                    in_=osb[:],
                )

            # ---------------- schedule ----------------
            emit_xin(0, 0)
            emit_xin(0, 1)
            for j in range(NBATCH):
                emit_batch(0, j)
            emit_finalize(0)
            # sample-1 compute overlapped with sample-0 normalize+store.
            # Pool owns the in-window normalizes; store DMAs ride qSP
            # (input queue is past its last transfer by then).
            s0_chunks = [(a, j) for j in range(NBATCH) for a in range(2)]
            for j in range(NBATCH):
                emit_batch(1, j)
                inw_eng = ["dve", "dve", "act", "dve", "dve", "act"]
                for a, jj in s0_chunks[2 * j:min(2 * j + 2, 6)]:
                    emit_norm(0, a, jj, inw_eng[2 * j + (1 if a else 0)], nc.scalar)
            emit_finalize(1)
            # tail: the two deferred s0 chunks ride ACT; sample-1 chunks ride
            # DVE 4x ops. DMAs alternate across both HWDGE queues.
            s1_eng = ["dve"] * 8
            tail = [(0, a, j, "act") for a, j in s0_chunks[6:]] + [
                (1, a, j, e) for (a, j), e in zip(s0_chunks, s1_eng)]
            dmas = [nc.sync, nc.scalar] * 5
            for i, (s, a, j, eng) in enumerate(tail):
                emit_norm(s, a, j, eng, dmas[i])

    nc.compile()
    return nc


def _get_nc():
    if "nc" not in _NC_CACHE:
        _NC_CACHE["nc"] = _build_nc()
    return _NC_CACHE["nc"]


def _make_in_maps(x, W, gamma, beta):
    bf16 = ml_dtypes.bfloat16
    x = np.asarray(x, dtype=np.float32)
    W = np.asarray(W, dtype=np.float32)
    gamma = np.asarray(gamma, dtype=np.float32)
    beta = np.asarray(beta, dtype=np.float32)

    # [B, N, F] -> [B, F, N] bf16 (per-sample transpose on host)
    xt = x.swapaxes(1, 2).astype(bf16)
    wt = np.ascontiguousarray(W.T).astype(bf16)
    return [
        {
            "x": xt[i * SPB:(i + 1) * SPB],
            "wt": wt, "gamma": gamma, "beta": beta,
        }
        for i in range(CORES)
    ]


def kernel(x, W, b, gamma, beta):
    from concourse.bass_utils import run_bass_kernel_spmd

    nc = _get_nc()
    in_maps = _make_in_maps(x, W, gamma, beta)
    try:
        res = run_bass_kernel_spmd(nc, in_maps, core_ids=list(range(CORES)))
    except Exception:
        # One retry: a previous crashed run can leave a core wedged.
        res = run_bass_kernel_spmd(nc, in_maps, core_ids=list(range(CORES)))
    out = np.stack([res.results[i]["out"] for i in range(CORES)])
    # [CORES, SPB, F, N] -> [B, N, F] f32
    return out.reshape(B, F, N).swapaxes(1, 2).astype(np.float32)


if __name__ == "__main__":
    rng = np.random.default_rng(0)
    x = rng.standard_normal((B, N, F), dtype=np.float32)
    W = ((rng.random((F, F), dtype=np.float32) - 0.5) / 8).astype(np.float32)
    b = ((rng.random(F, dtype=np.float32) - 0.5) / 8).astype(np.float32)
    gamma = np.ones(F, np.float32)
    beta = np.zeros(F, np.float32)
    out = kernel(x=x, W=W, b=b, gamma=gamma, beta=beta)
    y = x @ W.T + b
    mean = y.mean(axis=1, keepdims=True)
    var = ((y - mean) ** 2).mean(axis=1, keepdims=True)
    ref = (y - mean) / np.sqrt(var + EPS) * gamma + beta
    err = np.abs(out - ref).max()
    print("maxabs err:", err, "rel:", err / np.abs(ref).max())


# revision 21
# speedup vs baseline: 1.0234x; 1.0234x over previous
"""Trainium2 Bass kernel for nn_FC_89094801588783.

Computes, for x[B=16, N=8192, Fin=256], W[256,256], b[256], gamma[256], beta[256]:
    y = x @ W.T + b                       (per-token Linear)
    per-sample BatchNorm over N (biased var), then gamma/beta affine.

Key structural choices vs a straightforward port:
  - bf16 end-to-end on device (x, W, y storage, output). The harness gate is
    rel_err < 2e-2; the bf16 pipeline measures ~5e-3. Halves HBM traffic.
  - The host pre-transposes x to [sample, Fin, N] and post-transposes the
    output, so the device never runs a PE transpose: the only PE work is the
    W-stationary matmul producing y^T[Fout, tok] directly.
  - The Linear bias b cancels exactly in BatchNorm (mean subtraction), so it
    is never sent to the device.
  - Per 2048-token batch: 16 matmuls accumulate into 8 PSUM banks; ACT
    evacuates PSUM->SBUF bf16 with accum_out (per-feature sum -> mean); DVE
    squares the bf16 y with accum_out (sum sq -> var). The normalize
    (y*k + shift) runs as DVE tensor_scalar ops, which hit the 4x DVE mode
    on all-bf16 operands (~0.8us per [128,2048] vs ~2us elsewhere), with a
    couple on ACT to balance the queues. GpSimd is kept off the big ops:
    concurrent Pool traffic poisons the shared SBUF ports and halves DVE
    throughput.
  - The per-sample finalize (mean/var -> k, shift) runs on DVE in its idle
    gap between the two samples' windows, with a single Sqrt hop on ACT.
  - DMAs are spread over both HWDGE queues (inputs on qSP; consts, store
    DMAs and half the tail on qAct) so transfers overlap instead of
    serializing on one ring.

Sharding: data-parallel over B across 8 NeuronCores (2 samples per core).
"""
import sys

sys.path.insert(0, "/opt/trn_rl_repo")

import numpy as np
import ml_dtypes

_NC_CACHE = {}

B, N, F = 16, 8192, 256
CORES = 8
SPB = B // CORES          # samples per core = 2
P = 128
NBATCH = 4                # token batches per sample
BT = N // NBATCH          # tokens per batch = 2048
NG = BT // 512            # 512-token PSUM groups per batch = 4
EPS = 1e-5
INV_N = 1.0 / N


def _build_nc():
    import concourse.bacc as bacc
    import concourse.tile as tile
    from concourse import mybir

    f32 = mybir.dt.float32
    bf16 = mybir.dt.bfloat16
    AF = mybir.ActivationFunctionType
    OP = mybir.AluOpType
    AX = mybir.AxisListType

    nc = bacc.Bacc("TRN2")
    x_d = nc.dram_tensor("x", [SPB, F, N], bf16, kind="ExternalInput")
    wt_d = nc.dram_tensor("wt", [F, F], bf16, kind="ExternalInput")
    g_d = nc.dram_tensor("gamma", [F], f32, kind="ExternalInput")
    be_d = nc.dram_tensor("beta", [F], f32, kind="ExternalInput")
    out_d = nc.dram_tensor("out", [SPB, F, N], bf16, kind="ExternalOutput")

    with tile.TileContext(nc) as tc:
        with (
            tc.tile_pool(name="consts", bufs=1) as consts,
            tc.tile_pool(name="xin", bufs=3) as xin,
            tc.tile_pool(name="xin0", bufs=4) as xin0,
            tc.tile_pool(name="ystore", bufs=1) as ystore,
            tc.tile_pool(name="acc", bufs=1) as accp,
            tc.tile_pool(name="fin", bufs=1) as finp,
            tc.tile_pool(name="scr", bufs=2) as scr,
            tc.tile_pool(name="outp", bufs=10) as outp,
            tc.tile_pool(name="ps", bufs=2, space="PSUM") as ps,
        ):
            # ---------------- constants (qAct queue; x input rides qSP) ----
            wT = consts.tile([P, 2, F], bf16)
            nc.scalar.dma_start(out=wT[:], in_=wt_d.rearrange("(c p) o -> p c o", p=P))
            g_col = consts.tile([P, 2], f32)
            nc.scalar.dma_start(out=g_col[:], in_=g_d.rearrange("(h p) -> p h", p=P))
            be_col = consts.tile([P, 2], f32)
            nc.scalar.dma_start(out=be_col[:], in_=be_d.rearrange("(h p) -> p h", p=P))

            # per-(sample, a) persistent state
            y_sb = [[None] * 2 for _ in range(SPB)]
            sums_t = [[None] * 2 for _ in range(SPB)]
            sq_t = [[None] * 2 for _ in range(SPB)]
            k_col = [[None] * 2 for _ in range(SPB)]
            sh_col = [[None] * 2 for _ in range(SPB)]
            for s in range(SPB):
                for a in range(2):
                    y_sb[s][a] = ystore.tile(
                        [P, NBATCH, BT], bf16, tag=f"y{s}{a}", name=f"y{s}{a}"
                    )
                    sums_t[s][a] = accp.tile(
                        [P, NBATCH], f32, tag=f"sm{s}{a}", name=f"sm{s}{a}")
                    sq_t[s][a] = accp.tile(
                        [P, NBATCH], f32, tag=f"sq{s}{a}", name=f"sq{s}{a}")

            xin_t = {}

            def emit_xin(s, j):
                if s == 0 and j == 0:
                    # first batch in 4 chunks so the first matmul starts early
                    parts = []
                    for g in range(NG):
                        t = xin0.tile([P, 2, 512], bf16, tag="x0", name=f"x0{g}")
                        eng = nc.sync if g % 2 == 0 else nc.scalar
                        eng.dma_start(
                            out=t[:],
                            in_=x_d[0, :, g * 512:(g + 1) * 512].rearrange(
                                "(c p) t -> p c t", p=P),
                        )
                        parts.append(t)
                    xin_t[(s, j)] = parts
                else:
                    t = xin.tile([P, 2, BT], bf16, tag="x")
                    nc.sync.dma_start(
                        out=t[:],
                        in_=x_d[s, :, j * BT:(j + 1) * BT].rearrange(
                            "(c p) t -> p c t", p=P),
                    )
                    xin_t[(s, j)] = t

            def emit_batch(s, j):
                x_t = xin_t.pop((s, j))
                nxt = s * NBATCH + j + 2
                if nxt < SPB * NBATCH:
                    emit_xin(nxt // NBATCH, nxt % NBATCH)

                pst = [ps.tile([P, NG, 512], f32, tag="ps", name=f"ps{s}{j}{a}")
                       for a in range(2)]
                # c outermost so 4 consecutive matmuls share one stationary
                for c in range(2):
                    for a in range(2):
                        for g in range(NG):
                            rhs = (x_t[g][:, c, :] if isinstance(x_t, list)
                                   else x_t[:, c, g * 512:(g + 1) * 512])
                            nc.tensor.matmul(
                                pst[a][:, g, :],
                                wT[:, c, a * P:(a + 1) * P],
                                rhs,
                                start=(c == 0), stop=(c == 1),
                            )
                for a in range(2):
                    # PSUM -> SBUF bf16; accumulator gives sum(y) per feature
                    nc.scalar.activation(
                        out=y_sb[s][a][:, j, :],
                        in_=pst[a].rearrange("p g t -> p (g t)"),
                        func=AF.Copy,
                        accum_out=sums_t[s][a][:, j:j + 1],
                    )
                for a in range(2):
                    # sum(y^2) from the bf16 y (walrus allows only one PSUM
                    # operand per DVE op, so this reads the evac result)
                    sq_scr = scr.tile([P, BT], bf16, tag="scr")
                    nc.vector.scalar_tensor_tensor(
                        out=sq_scr[:],
                        in0=y_sb[s][a][:, j, :],
                        scalar=1.0,
                        in1=y_sb[s][a][:, j, :],
                        op0=OP.mult,
                        op1=OP.mult,
                        accum_out=sq_t[s][a][:, j:j + 1],
                    )

            def emit_finalize(s):
                # Runs in the DVE gap between the two samples' compute
                # windows; single Sqrt hop on ACT (Rsqrt is banned).
                for a in range(2):
                    S = finp.tile([P, 1], f32, tag=f"S{s}{a}", name=f"S{s}{a}")
                    nc.vector.tensor_reduce(
                        out=S[:], in_=sums_t[s][a][:], axis=AX.X, op=OP.add)
                    Q = finp.tile([P, 1], f32, tag=f"Q{s}{a}", name=f"Q{s}{a}")
                    nc.vector.tensor_reduce(
                        out=Q[:], in_=sq_t[s][a][:], axis=AX.X, op=OP.add)
                    mean = finp.tile([P, 1], f32, tag=f"mn{s}{a}", name=f"mn{s}{a}")
                    nc.vector.tensor_scalar_mul(out=mean[:], in0=S[:], scalar1=INV_N)
                    msq = finp.tile([P, 1], f32, tag=f"ms{s}{a}", name=f"ms{s}{a}")
                    nc.vector.tensor_mul(out=msq[:], in0=mean[:], in1=mean[:])
                    vare = finp.tile([P, 1], f32, tag=f"vr{s}{a}", name=f"vr{s}{a}")
                    nc.vector.tensor_scalar(
                        out=vare[:], in0=Q[:], scalar1=INV_N, scalar2=EPS,
                        op0=OP.mult, op1=OP.add,
                    )
                    nc.vector.tensor_sub(out=vare[:], in0=vare[:], in1=msq[:])
                    ivar = finp.tile([P, 1], f32, tag=f"iv{s}{a}", name=f"iv{s}{a}")
                    nc.vector.reciprocal(out=ivar[:], in_=vare[:])
                    k0 = finp.tile([P, 1], f32, tag=f"k0{s}{a}", name=f"k0{s}{a}")
                    nc.scalar.activation(
                        out=k0[:], in_=ivar[:], func=AF.Sqrt, bias=0.0, scale=1.0)
                    k = finp.tile([P, 1], f32, tag=f"k{s}{a}", name=f"k{s}{a}")
                    nc.vector.tensor_mul(out=k[:], in0=k0[:], in1=g_col[:, a:a + 1])
                    tmp = finp.tile([P, 1], f32, tag=f"tp{s}{a}", name=f"tp{s}{a}")
                    nc.vector.tensor_mul(out=tmp[:], in0=mean[:], in1=k[:])
                    sh = finp.tile([P, 1], f32, tag=f"sh{s}{a}", name=f"sh{s}{a}")
                    nc.vector.tensor_sub(
                        out=sh[:], in0=be_col[:, a:a + 1], in1=tmp[:])
                    k_col[s][a] = k
                    sh_col[s][a] = sh

            def emit_norm(s, a, j, eng, dma_eng):
                osb = outp.tile([P, BT], bf16, tag="o")
                if eng == "act":
                    nc.scalar.activation(
                        out=osb[:], in_=y_sb[s][a][:, j, :], func=AF.Identity,
                        bias=sh_col[s][a][:], scale=k_col[s][a][:],
                    )
                else:
                    e = nc.vector if eng == "dve" else nc.gpsimd
                    e.tensor_scalar(
                        out=osb[:], in0=y_sb[s][a][:, j, :],
                        scalar1=k_col[s][a][:], scalar2=sh_col[s][a][:],
                        op0=OP.mult, op1=OP.add,
                    )
                dma_eng.dma_start(
                    out=out_d[s, a * P:(a + 1) * P, j * BT:(j + 1) * BT],
                    in_=osb[:],
                )

            # ---------------- schedule ----------------
            emit_xin(0, 0)
            emit_xin(0, 1)
            for j in range(NBATCH):
                emit_batch(0, j)
            emit_finalize(0)
            # sample-1 compute overlapped with sample-0 normalize+store.
            # Pool owns the in-window normalizes; store DMAs ride qSP
            # (input queue is past its last transfer by then).
            s0_chunks = [(a, j) for j in range(NBATCH) for a in range(2)]
            for j in range(NBATCH):
                emit_batch(1, j)
                inw_eng = ["dve", "dve", "act", "dve", "dve", "act"]
                for a, jj in s0_chunks[2 * j:min(2 * j + 2, 6)]:
                    emit_norm(0, a, jj, inw_eng[2 * j + (1 if a else 0)], nc.scalar)
            emit_finalize(1)
            # tail: the two deferred s0 chunks ride ACT; sample-1 chunks ride
            # DVE 4x ops. DMAs alternate across both HWDGE queues.
            s1_eng = ["dve"] * 8
            tail = [(0, a, j, "act") for a, j in s0_chunks[6:]] + [
                (1, a, j, e) for (a, j), e in zip(s0_chunks, s1_eng)]
            dmas = [nc.sync, nc.scalar] * 5
            for i, (s, a, j, eng) in enumerate(tail):
                emit_norm(s, a, j, eng, dmas[i])

    nc.compile()
    return nc


def _get_nc():
    if "nc" not in _NC_CACHE:
        _NC_CACHE["nc"] = _build_nc()
    return _NC_CACHE["nc"]


def _make_in_maps(x, W, gamma, beta):
    bf16 = ml_dtypes.bfloat16
    x = np.asarray(x, dtype=np.float32)
    W = np.asarray(W, dtype=np.float32)
    gamma = np.asarray(gamma, dtype=np.float32)
    beta = np.asarray(beta, dtype=np.float32)

    # [B, N, F] -> [B, F, N] bf16 (per-sample transpose on host)
    xt = x.swapaxes(1, 2).astype(bf16)
    wt = np.ascontiguousarray(W.T).astype(bf16)
    return [
        {
            "x": xt[i * SPB:(i + 1) * SPB],
            "wt": wt, "gamma": gamma, "beta": beta,
        }
        for i in range(CORES)
    ]


def kernel(x, W, b, gamma, beta):
    from concourse.bass_utils import run_bass_kernel_spmd

    nc = _get_nc()
    in_maps = _make_in_maps(x, W, gamma, beta)
    try:
        res = run_bass_kernel_spmd(nc, in_maps, core_ids=list(range(CORES)))
    except Exception:
        # One retry: a previous crashed run can leave a core wedged.
        res = run_bass_kernel_spmd(nc, in_maps, core_ids=list(range(CORES)))
    out = np.stack([res.results[i]["out"] for i in range(CORES)])
    # [CORES, SPB, F, N] -> [B, N, F] f32
    return out.reshape(B, F, N).swapaxes(1, 2).astype(np.float32)


if __name__ == "__main__":
    rng = np.random.default_rng(0)
    x = rng.standard_normal((B, N, F), dtype=np.float32)
    W = ((rng.random((F, F), dtype=np.float32) - 0.5) / 8).astype(np.float32)
    b = ((rng.random(F, dtype=np.float32) - 0.5) / 8).astype(np.float32)
    gamma = np.ones(F, np.float32)
    beta = np.zeros(F, np.float32)
    out = kernel(x=x, W=W, b=b, gamma=gamma, beta=beta)
    y = x @ W.T + b
    mean = y.mean(axis=1, keepdims=True)
    var = ((y - mean) ** 2).mean(axis=1, keepdims=True)
    ref = (y - mean) / np.sqrt(var + EPS) * gamma + beta
    err = np.abs(out - ref).max()
    print("maxabs err:", err, "rel:", err / np.abs(ref).max())


# revision 23
# speedup vs baseline: 1.1528x; 1.1264x over previous
"""Trainium2 Bass kernel for nn_FC_89094801588783.

Computes, for x[B=16, N=8192, Fin=256], W[256,256], b[256], gamma[256], beta[256]:
    y = x @ W.T + b                       (per-token Linear)
    per-sample BatchNorm over N (biased var), then gamma/beta affine.

Key structural choices vs a straightforward port:
  - bf16 end-to-end on device (x, W, y storage, output). The harness gate is
    rel_err < 2e-2; the bf16 pipeline measures ~5e-3. Halves HBM traffic.
  - The host pre-transposes x to [sample, Fin, N] and post-transposes the
    output, so the device never runs a PE transpose: the only PE work is the
    W-stationary matmul producing y^T[Fout, tok] directly.
  - The Linear bias b cancels exactly in BatchNorm (mean subtraction), so it
    is never sent to the device.
  - Per 2048-token batch: 16 matmuls accumulate into 8 PSUM banks; ACT
    evacuates PSUM->SBUF bf16 with accum_out (per-feature sum -> mean); DVE
    squares the bf16 y with accum_out (sum sq -> var). The normalize
    (y*k + shift) runs as DVE tensor_scalar ops, which hit the 4x DVE mode
    on all-bf16 operands (~0.8us per [128,2048] vs ~2us elsewhere), with a
    couple on ACT to balance the queues. GpSimd is kept off the big ops:
    concurrent Pool traffic poisons the shared SBUF ports and halves DVE
    throughput.
  - The per-sample finalize (mean/var -> k, shift) runs on DVE in its idle
    gap between the two samples' windows, with a single Sqrt hop on ACT.
  - DMAs are spread over both HWDGE queues (inputs on qSP; consts, store
    DMAs and half the tail on qAct) so transfers overlap instead of
    serializing on one ring.

Sharding: data-parallel over B across 8 NeuronCores (2 samples per core).
"""
import sys

sys.path.insert(0, "/opt/trn_rl_repo")

import numpy as np
import ml_dtypes

_NC_CACHE = {}

B, N, F = 16, 8192, 256
CORES = 8
SPB = B // CORES          # samples per core = 2
P = 128
NBATCH = 4                # token batches per sample
BT = N // NBATCH          # tokens per batch = 2048
NG = BT // 512            # 512-token PSUM groups per batch = 4
EPS = 1e-5
INV_N = 1.0 / N


def _build_nc():
    import concourse.bacc as bacc
    import concourse.tile as tile
    from concourse import mybir

    f32 = mybir.dt.float32
    bf16 = mybir.dt.bfloat16
    AF = mybir.ActivationFunctionType
    OP = mybir.AluOpType
    AX = mybir.AxisListType

    nc = bacc.Bacc("TRN2")
    x_d = nc.dram_tensor("x", [SPB, F, N], bf16, kind="ExternalInput")
    wt_d = nc.dram_tensor("wt", [F, F], bf16, kind="ExternalInput")
    g_d = nc.dram_tensor("gamma", [F], f32, kind="ExternalInput")
    be_d = nc.dram_tensor("beta", [F], f32, kind="ExternalInput")
    out_d = nc.dram_tensor("out", [SPB, F, N], bf16, kind="ExternalOutput")

    with tile.TileContext(nc) as tc:
        with (
            tc.tile_pool(name="consts", bufs=1) as consts,
            tc.tile_pool(name="xin", bufs=3) as xin,
            tc.tile_pool(name="xin0", bufs=4) as xin0,
            tc.tile_pool(name="ystore", bufs=1) as ystore,
            tc.tile_pool(name="acc", bufs=1) as accp,
            tc.tile_pool(name="fin", bufs=1) as finp,
            tc.tile_pool(name="scr", bufs=2) as scr,
            tc.tile_pool(name="outp", bufs=10) as outp,
            tc.tile_pool(name="ps", bufs=2, space="PSUM") as ps,
        ):
            # ---------------- constants (qAct queue; x input rides qSP) ----
            wT = consts.tile([P, 2, F], bf16)
            nc.scalar.dma_start(out=wT[:], in_=wt_d.rearrange("(c p) o -> p c o", p=P))
            g_col = consts.tile([P, 2], f32)
            nc.scalar.dma_start(out=g_col[:], in_=g_d.rearrange("(h p) -> p h", p=P))
            be_col = consts.tile([P, 2], f32)
            nc.scalar.dma_start(out=be_col[:], in_=be_d.rearrange("(h p) -> p h", p=P))

            # per-(sample, a) persistent state
            y_sb = [[None] * 2 for _ in range(SPB)]
            sums_t = [[None] * 2 for _ in range(SPB)]
            sq_t = [[None] * 2 for _ in range(SPB)]
            k_col = [[None] * 2 for _ in range(SPB)]
            sh_col = [[None] * 2 for _ in range(SPB)]
            for s in range(SPB):
                for a in range(2):
                    y_sb[s][a] = ystore.tile(
                        [P, NBATCH, BT], bf16, tag=f"y{s}{a}", name=f"y{s}{a}"
                    )
                    sums_t[s][a] = accp.tile(
                        [P, NBATCH], f32, tag=f"sm{s}{a}", name=f"sm{s}{a}")
                    sq_t[s][a] = accp.tile(
                        [P, NBATCH], f32, tag=f"sq{s}{a}", name=f"sq{s}{a}")

            xin_t = {}

            def emit_xin(s, j):
                if s == 0 and j == 0:
                    # first batch in 4 chunks so the first matmul starts early
                    parts = []
                    for g in range(NG):
                        t = xin0.tile([P, 2, 512], bf16, tag="x0", name=f"x0{g}")
                        nc.sync.dma_start(
                            out=t[:],
                            in_=x_d[0, :, g * 512:(g + 1) * 512].rearrange(
                                "(c p) t -> p c t", p=P),
                        )
                        parts.append(t)
                    xin_t[(s, j)] = parts
                else:
                    t = xin.tile([P, 2, BT], bf16, tag="x")
                    nc.sync.dma_start(
                        out=t[:],
                        in_=x_d[s, :, j * BT:(j + 1) * BT].rearrange(
                            "(c p) t -> p c t", p=P),
                    )
                    xin_t[(s, j)] = t

            def emit_batch(s, j):
                x_t = xin_t.pop((s, j))
                nxt = s * NBATCH + j + 2
                if nxt < SPB * NBATCH:
                    emit_xin(nxt // NBATCH, nxt % NBATCH)

                pst = [ps.tile([P, NG, 512], f32, tag="ps", name=f"ps{s}{j}{a}")
                       for a in range(2)]
                # c outermost so 4 consecutive matmuls share one stationary
                for c in range(2):
                    for a in range(2):
                        for g in range(NG):
                            rhs = (x_t[g][:, c, :] if isinstance(x_t, list)
                                   else x_t[:, c, g * 512:(g + 1) * 512])
                            nc.tensor.matmul(
                                pst[a][:, g, :],
                                wT[:, c, a * P:(a + 1) * P],
                                rhs,
                                start=(c == 0), stop=(c == 1),
                            )
                for a in range(2):
                    # PSUM -> SBUF bf16; accumulator gives sum(y) per feature
                    nc.scalar.activation(
                        out=y_sb[s][a][:, j, :],
                        in_=pst[a].rearrange("p g t -> p (g t)"),
                        func=AF.Copy,
                        accum_out=sums_t[s][a][:, j:j + 1],
                    )
                for a in range(2):
                    # sum(y^2) from the bf16 y (walrus allows only one PSUM
                    # operand per DVE op, so this reads the evac result)
                    sq_scr = scr.tile([P, BT], bf16, tag="scr")
                    nc.vector.scalar_tensor_tensor(
                        out=sq_scr[:],
                        in0=y_sb[s][a][:, j, :],
                        scalar=1.0,
                        in1=y_sb[s][a][:, j, :],
                        op0=OP.mult,
                        op1=OP.mult,
                        accum_out=sq_t[s][a][:, j:j + 1],
                    )

            def emit_finalize(s):
                # Runs in the DVE gap between the two samples' compute
                # windows; single Sqrt hop on ACT (Rsqrt is banned).
                for a in range(2):
                    S = finp.tile([P, 1], f32, tag=f"S{s}{a}", name=f"S{s}{a}")
                    nc.vector.tensor_reduce(
                        out=S[:], in_=sums_t[s][a][:], axis=AX.X, op=OP.add)
                    Q = finp.tile([P, 1], f32, tag=f"Q{s}{a}", name=f"Q{s}{a}")
                    nc.vector.tensor_reduce(
                        out=Q[:], in_=sq_t[s][a][:], axis=AX.X, op=OP.add)
                    mean = finp.tile([P, 1], f32, tag=f"mn{s}{a}", name=f"mn{s}{a}")
                    nc.vector.tensor_scalar_mul(out=mean[:], in0=S[:], scalar1=INV_N)
                    msq = finp.tile([P, 1], f32, tag=f"ms{s}{a}", name=f"ms{s}{a}")
                    nc.vector.tensor_mul(out=msq[:], in0=mean[:], in1=mean[:])
                    vare = finp.tile([P, 1], f32, tag=f"vr{s}{a}", name=f"vr{s}{a}")
                    nc.vector.tensor_scalar(
                        out=vare[:], in0=Q[:], scalar1=INV_N, scalar2=EPS,
                        op0=OP.mult, op1=OP.add,
                    )
                    nc.vector.tensor_sub(out=vare[:], in0=vare[:], in1=msq[:])
                    ivar = finp.tile([P, 1], f32, tag=f"iv{s}{a}", name=f"iv{s}{a}")
                    nc.vector.reciprocal(out=ivar[:], in_=vare[:])
                    k0 = finp.tile([P, 1], f32, tag=f"k0{s}{a}", name=f"k0{s}{a}")
                    nc.scalar.activation(
                        out=k0[:], in_=ivar[:], func=AF.Sqrt, bias=0.0, scale=1.0)
                    k = finp.tile([P, 1], f32, tag=f"k{s}{a}", name=f"k{s}{a}")
                    nc.vector.tensor_mul(out=k[:], in0=k0[:], in1=g_col[:, a:a + 1])
                    tmp = finp.tile([P, 1], f32, tag=f"tp{s}{a}", name=f"tp{s}{a}")
                    nc.vector.tensor_mul(out=tmp[:], in0=mean[:], in1=k[:])
                    sh = finp.tile([P, 1], f32, tag=f"sh{s}{a}", name=f"sh{s}{a}")
                    nc.vector.tensor_sub(
                        out=sh[:], in0=be_col[:, a:a + 1], in1=tmp[:])
                    k_col[s][a] = k
                    sh_col[s][a] = sh

            def emit_norm(s, a, j, eng, dma_eng):
                osb = outp.tile([P, BT], bf16, tag="o")
                if eng == "act":
                    nc.scalar.activation(
                        out=osb[:], in_=y_sb[s][a][:, j, :], func=AF.Identity,
                        bias=sh_col[s][a][:], scale=k_col[s][a][:],
                    )
                else:
                    e = nc.vector if eng == "dve" else nc.gpsimd
                    e.tensor_scalar(
                        out=osb[:], in0=y_sb[s][a][:, j, :],
                        scalar1=k_col[s][a][:], scalar2=sh_col[s][a][:],
                        op0=OP.mult, op1=OP.add,
                    )
                dma_eng.dma_start(
                    out=out_d[s, a * P:(a + 1) * P, j * BT:(j + 1) * BT],
                    in_=osb[:],
                )

            # ---------------- schedule ----------------
            emit_xin(0, 0)
            emit_xin(0, 1)
            for j in range(NBATCH):
                emit_batch(0, j)
            emit_finalize(0)
            # sample-1 compute overlapped with sample-0 normalize+store.
            # Pool owns the in-window normalizes; store DMAs ride qSP
            # (input queue is past its last transfer by then).
            s0_chunks = [(a, j) for j in range(NBATCH) for a in range(2)]
            for j in range(NBATCH):
                emit_batch(1, j)
                inw_eng = ["dve", "dve", "act", "dve", "dve", "act"]
                for a, jj in s0_chunks[2 * j:min(2 * j + 2, 6)]:
                    emit_norm(0, a, jj, inw_eng[2 * j + (1 if a else 0)], nc.scalar)
            emit_finalize(1)
            # tail: the two deferred s0 chunks ride ACT; sample-1 chunks ride
            # DVE 4x ops. DMAs alternate across both HWDGE queues.
            s1_eng = ["dve"] * 8
            tail = [(0, a, j, "act") for a, j in s0_chunks[6:]] + [
                (1, a, j, e) for (a, j), e in zip(s0_chunks, s1_eng)]
            dmas = [nc.sync, nc.scalar, nc.gpsimd, nc.sync, nc.scalar,
                    nc.gpsimd, nc.sync, nc.scalar, nc.gpsimd, nc.sync]
            for i, (s, a, j, eng) in enumerate(tail):
                emit_norm(s, a, j, eng, dmas[i])

    nc.compile()
    return nc


def _get_nc():
    if "nc" not in _NC_CACHE:
        _NC_CACHE["nc"] = _build_nc()
    return _NC_CACHE["nc"]


def _make_in_maps(x, W, gamma, beta):
    bf16 = ml_dtypes.bfloat16
    x = np.asarray(x, dtype=np.float32)
    W = np.asarray(W, dtype=np.float32)
    gamma = np.asarray(gamma, dtype=np.float32)
    beta = np.asarray(beta, dtype=np.float32)

    # [B, N, F] -> [B, F, N] bf16 (per-sample transpose on host)
    xt = x.swapaxes(1, 2).astype(bf16)
    wt = np.ascontiguousarray(W.T).astype(bf16)
    return [
        {
            "x": xt[i * SPB:(i + 1) * SPB],
            "wt": wt, "gamma": gamma, "beta": beta,
        }
        for i in range(CORES)
    ]


def kernel(x, W, b, gamma, beta):
    from concourse.bass_utils import run_bass_kernel_spmd

    nc = _get_nc()
    in_maps = _make_in_maps(x, W, gamma, beta)
    try:
        res = run_bass_kernel_spmd(nc, in_maps, core_ids=list(range(CORES)))
    except Exception:
        # One retry: a previous crashed run can leave a core wedged.
        res = run_bass_kernel_spmd(nc, in_maps, core_ids=list(range(CORES)))
    out = np.stack([res.results[i]["out"] for i in range(CORES)])
    # [CORES, SPB, F, N] -> [B, N, F] f32
    return out.reshape(B, F, N).swapaxes(1, 2).astype(np.float32)


if __name__ == "__main__":
    rng = np.random.default_rng(0)
    x = rng.standard_normal((B, N, F), dtype=np.float32)
    W = ((rng.random((F, F), dtype=np.float32) - 0.5) / 8).astype(np.float32)
    b = ((rng.random(F, dtype=np.float32) - 0.5) / 8).astype(np.float32)
    gamma = np.ones(F, np.float32)
    beta = np.zeros(F, np.float32)
    out = kernel(x=x, W=W, b=b, gamma=gamma, beta=beta)
    y = x @ W.T + b
    mean = y.mean(axis=1, keepdims=True)
    var = ((y - mean) ** 2).mean(axis=1, keepdims=True)
    ref = (y - mean) / np.sqrt(var + EPS) * gamma + beta
    err = np.abs(out - ref).max()
    print("maxabs err:", err, "rel:", err / np.abs(ref).max())
